# revision 4
# baseline (speedup 1.0000x reference)
"""Trainium2 Bass kernel for a single causal attention head.

  x:  [32, 1024, 768] f32, Wq/Wk/Wv: [64, 768] f32
  out[b,q,:] = softmax_k(causal(Q K^T / 8)) @ V,  Q = x Wq^T etc.

Sharding: data-parallel over batch — 4 batches per core on 8 cores,
weights replicated. Everything else on-device:

per batch b (all on one NeuronCore):
  1. x^T [768, 1024] via 48 PE transposes (contraction dim must sit on
     partitions for every matmul, so x must be transposed once).
  2. [Q^T; K^T] [128, 1024] in one PSUM pass (Wq/Wk packed into one
     128-wide stationary), V^T [64, 1024] in a second pass.
  3. S^T = K^T.T Q^T per 128-row k-block (only causal blocks), exp on
     the scalar engine, diagonal mask via gpsimd affine_select.
  4. out^T|denom [65, 1024] accumulated as [V | ones].T @ E — the ones
     column makes row 64 the softmax denominators (softmax over k is a
     partition-dim reduction here, so it rides the matmul for free).
  5. transpose 128-col blocks back, scale by 1/denom, DMA out.
"""

import os
import sys
import numpy as np

B_FULL = 32
N_CORES = 8
B_CORE = B_FULL // N_CORES
T, C, D = 1024, 768, 64
TT = T // 128  # 8
CC = C // 128  # 6
SCALE = 1.0 / np.sqrt(D)

# Matmul input dtype tag: float32r streams at ~4x the rate of float32 on
# the PE for free dims >= 256. Flip with USE_F32R=0 to fall back.
USE_F32R = os.environ.get("USE_F32R", "1") == "1"

_cache = {}


def _seg512(q0, q1):
    """Split [q0, q1) at the 512 boundary (PSUM bank / fp32 moving-operand
    limit). Every segment must live inside one 2KB PSUM bank."""
    segs = []
    while q0 < q1:
        q_end = min(q1, (q0 // 512 + 1) * 512)
        segs.append((q0, q_end))
        q0 = q_end
    return segs


def _build():
    from contextlib import ExitStack

    import concourse.bass as bass
    import concourse.tile as tile
    from concourse import bacc, mybir
    from concourse.bass import ts
    from concourse.masks import make_identity

    f32 = mybir.dt.float32
    # Matmul-input tiles use float32r: walrus requires every producer of an
    # fp32r matmul operand to round its output, so the rounding happens at
    # the PSUM->SBUF copy / activation that writes the tile.
    rdt = mybir.dt.float32r if USE_F32R else f32
    nc = bacc.Bacc("TRN2", target_bir_lowering=False, debug=False)
    x = nc.dram_tensor("x", [B_CORE, T, C], f32, kind="ExternalInput").ap()
    wq = nc.dram_tensor("Wq", [D, C], f32, kind="ExternalInput").ap()
    wk = nc.dram_tensor("Wk", [D, C], f32, kind="ExternalInput").ap()
    wv = nc.dram_tensor("Wv", [D, C], f32, kind="ExternalInput").ap()
    y = nc.dram_tensor("y", [B_CORE, T, D], f32, kind="ExternalOutput").ap()

    with tile.TileContext(nc) as tc, ExitStack() as ctx:
        const = ctx.enter_context(tc.tile_pool(name="const", bufs=1))
        xpool = ctx.enter_context(tc.tile_pool(name="xload", bufs=2))
        xtp = ctx.enter_context(tc.tile_pool(name="xt", bufs=2))
        sb = ctx.enter_context(tc.tile_pool(name="sb", bufs=2))
        epool = ctx.enter_context(tc.tile_pool(name="e", bufs=3))
        ypool = ctx.enter_context(tc.tile_pool(name="yout", bufs=2))
        ps_big = ctx.enter_context(tc.tile_pool(name="ps_big", bufs=2, space="PSUM"))
        ps_acc = ctx.enter_context(tc.tile_pool(name="ps_acc", bufs=1, space="PSUM"))
        ps_sm = ctx.enter_context(tc.tile_pool(name="ps_sm", bufs=2, space="PSUM"))

        ident = const.tile([128, 128], f32, tag="ident")
        make_identity(nc, ident)
        # additive causal mask for the S^T diagonal block: 0 where k<=q
        # (p<=f), -1e9 where k>q, so exp() zeroes masked entries
        dmask = const.tile([128, 128], f32, tag="dmask")
        nc.gpsimd.memset(dmask, 0.0)
        nc.gpsimd.affine_select(
            out=dmask,
            in_=dmask,
            compare_op=mybir.AluOpType.is_ge,
            fill=-1e9,
            base=0,
            pattern=[[1, 128]],
            channel_multiplier=-1,
        )

        # Weights: DMA in natural [64, 768] layout, PE-transpose the 128-col
        # chunks into stationary form. WQK packs [Wq^T | Wk^T] so one proj
        # pass emits Q^T on partitions 0:64 and K^T on 64:128.
        w_sb = const.tile([64, 3, C], f32, tag="wsb")
        nc.sync.dma_start(w_sb[:, 0, :], wq)
        nc.sync.dma_start(w_sb[:, 1, :], wk)
        nc.sync.dma_start(w_sb[:, 2, :], wv)
        WQK = const.tile([128, CC, 128], rdt, tag="wqk")
        WV = const.tile([128, CC, D], rdt, tag="wv")
        for j in range(CC):
            pw = ps_sm.tile([128, 128], f32, tag="ps_sm")
            nc.tensor.transpose(pw[:, 0:64], w_sb[:, 0, ts(j, 128)], ident[0:64, 0:64])
            nc.tensor.transpose(pw[:, 64:128], w_sb[:, 1, ts(j, 128)], ident[0:64, 0:64])
            nc.scalar.copy(WQK[:, j, :], pw)
            pw2 = ps_sm.tile([128, D], f32, tag="ps_sm")
            nc.tensor.transpose(pw2, w_sb[:, 2, ts(j, 128)], ident[0:64, 0:64])
            nc.vector.tensor_copy(WV[:, j, :], pw2)

        for b in range(B_CORE):
            # ---- load x[b] and build x^T ----
            x_sb = xpool.tile([128, TT, C], f32, tag="x")
            nc.sync.dma_start(x_sb, x[b].rearrange("(t p) c -> p t c", p=128))
            xT = xtp.tile([128, CC, T], rdt, tag="xT")
            for j in range(CC):
                for g in range(2):
                    pt = ps_sm.tile([128, 512], f32, tag="ps_sm")
                    for u in range(4):
                        t_i = g * 4 + u
                        nc.tensor.transpose(
                            pt[:, ts(u, 128)], x_sb[:, t_i, ts(j, 128)], ident
                        )
                    eng = nc.scalar if (2 * j + g) % 2 == 0 else nc.vector
                    if eng is nc.scalar:
                        eng.copy(xT[:, j, ts(g, 512)], pt)
                    else:
                        eng.tensor_copy(xT[:, j, ts(g, 512)], pt)

            # ---- projections ----
            qk_ps = ps_big.tile([128, T], f32, tag="ps_big")
            for h in range(2):
                for j in range(CC):
                    nc.tensor.matmul(
                        qk_ps[:, ts(h, 512)],
                        WQK[:, j, :],
                        xT[:, j, ts(h, 512)],
                        start=(j == 0),
                        stop=(j == CC - 1),
                    )
            vt_ps = ps_big.tile([64, T], f32, tag="ps_big")
            for h in range(2):
                for j in range(CC):
                    nc.tensor.matmul(
                        vt_ps[:, ts(h, 512)],
                        WV[:, j, :],
                        xT[:, j, ts(h, 512)],
                        start=(j == 0),
                        stop=(j == CC - 1),
                    )

            QK_sb = sb.tile([128, T], rdt, tag="qksb")
            nc.scalar.copy(QK_sb[:, 0:512], qk_ps[:, 0:512])
            nc.vector.tensor_copy(QK_sb[:, 512:1024], qk_ps[:, 512:1024])
            # V^T rows 0:64 + a row of ones; transposed 128-blocks give the
            # [V | ones] stationary whose ones-column accumulates softmax
            # denominators during the AV matmul.
            VT_sb = sb.tile([65, T], f32, tag="vtsb")
            nc.scalar.copy(VT_sb[0:64, :], vt_ps)
            nc.vector.memset(VT_sb[64:65, :], 1.0)
            # Q^T must also sit on partitions 64:128: the S^T matmul's
            # stationary (K^T) lives there, and lhsT/rhs partition ranges
            # must match. Engines can't cross partitions; DMA can.
            QTd = sb.tile([128, T], rdt, tag="qtd")
            nc.sync.dma_start(QTd[64:128, :], QK_sb[0:64, :])

            Vp = sb.tile([128, TT, D + 1], rdt, tag="vp")
            for g in range(2):
                pv = ps_sm.tile([128, 4 * (D + 1)], f32, tag="ps_sm")
                for u in range(4):
                    k_i = g * 4 + u
                    nc.tensor.transpose(
                        pv[:, u * (D + 1) : (u + 1) * (D + 1)],
                        VT_sb[0:65, ts(k_i, 128)],
                        ident[0:65, 0:65],
                    )
                nc.vector.tensor_copy(
                    Vp[:, g * 4 : (g + 1) * 4, :],
                    pv.rearrange("p (a b) -> p a b", a=4),
                )

            # ---- attention: S^T blocks, exp, mask, AV accumulation ----
            o_ps = ps_acc.tile([65, T], f32, tag="ps_acc")
            for kt in range(TT):
                q0_blk = kt * 128
                st = ps_big.tile([128, T], f32, tag="ps_big")
                for (q0, q1) in _seg512(q0_blk, T):
                    nc.tensor.matmul(
                        st[:, q0:q1],
                        QK_sb[64:128, ts(kt, 128)],
                        QTd[64:128, q0:q1],
                        start=True,
                        stop=True,
                    )
                nc.vector.tensor_add(st[:, ts(kt, 128)], st[:, ts(kt, 128)], dmask)
                E = epool.tile([128, T], rdt, tag="e")
                nc.scalar.activation(
                    E[:, q0_blk:T],
                    st[:, q0_blk:T],
                    mybir.ActivationFunctionType.Exp,
                    scale=float(SCALE),
                )
                for (q0, q1) in _seg512(q0_blk, T):
                    last = (q1 == 512 and kt == 3) or (q1 == T and kt == TT - 1)
                    nc.tensor.matmul(
                        o_ps[:, q0:q1],
                        Vp[:, kt, :],
                        E[:, q0:q1],
                        start=(kt == 0),
                        stop=last,
                    )

            # ---- normalize + emit ----
            OT_sb = sb.tile([65, T], f32, tag="otsb")
            nc.scalar.copy(OT_sb[:, 0:512], o_ps[:, 0:512])
            nc.vector.tensor_copy(OT_sb[:, 512:1024], o_ps[:, 512:1024])
            y_sb = ypool.tile([128, TT, D], f32, tag="y")
            for qt in range(TT):
                po = ps_sm.tile([128, D + 1], f32, tag="ps_sm")
                nc.tensor.transpose(po, OT_sb[0:65, ts(qt, 128)], ident[0:65, 0:65])
                rec = sb.tile([128, 1], f32, tag="rec")
                nc.vector.reciprocal(rec, po[:, D : D + 1])
                nc.vector.tensor_scalar_mul(y_sb[:, qt, :], po[:, 0:D], rec)
            nc.sync.dma_start(y[b].rearrange("(t p) d -> p t d", p=128), y_sb)

    nc.compile()
    return nc


def _get_nc():
    if "nc" not in _cache:
        _cache["nc"] = _build()
    return _cache["nc"]


def run(inputs, trace=False, tmpdir=None):
    """Shard, run on 8 cores, gather. Returns (y_full, BassKernelResults)."""
    from concourse.bass_utils import run_bass_kernel_spmd

    x = np.asarray(inputs["x"], dtype=np.float32)
    wq = np.ascontiguousarray(np.asarray(inputs["Wq"], dtype=np.float32))
    wk = np.ascontiguousarray(np.asarray(inputs["Wk"], dtype=np.float32))
    wv = np.ascontiguousarray(np.asarray(inputs["Wv"], dtype=np.float32))
    assert x.shape == (B_FULL, T, C)

    nc = _get_nc()
    in_maps = [
        {
            "x": np.ascontiguousarray(x[i * B_CORE : (i + 1) * B_CORE]),
            "Wq": wq,
            "Wk": wk,
            "Wv": wv,
        }
        for i in range(N_CORES)
    ]
    kwargs = {}
    if trace:
        _install_trace_shim()
        kwargs = {"trace": True, "tmpdir": tmpdir}
    res = run_bass_kernel_spmd(nc, in_maps, list(range(N_CORES)), **kwargs)
    out = np.concatenate([res.results[i]["y"] for i in range(N_CORES)], axis=0)
    return out, res


def kernel(**inputs) -> np.ndarray:
    out, _ = run(inputs, trace=False)
    return out


def _install_trace_shim():
    """The image's antenv lacks axon_hooks; register the NTFF profile hook
    ourselves so run_bass_kernel_spmd(trace=True) works. Test-only path."""
    import contextlib
    import types

    try:
        from antenv.axon_hooks import get_axon_ntff_profile_hook  # noqa: F401

        return
    except ImportError:
        pass
    import antenv
    from trn_agent_boot.trn_boot import _ntff_profile_via_ctypes

    mod = types.ModuleType("antenv.axon_hooks")
    mod._hook = _ntff_profile_via_ctypes("/opt/axon/libaxon_pjrt.so")
    mod.set_axon_ntff_profile_hook = lambda h: setattr(mod, "_hook", h)
    mod.get_axon_ntff_profile_hook = lambda: mod._hook
    sys.modules["antenv.axon_hooks"] = mod
    antenv.axon_hooks = mod

    import concourse.bass_utils as bu

    bu.upload_artifacts = lambda tmpdir: tmpdir
